# revision 12
# baseline (speedup 1.0000x reference)
"""KimiLinear KDA decode step — Trainium2 Bass kernel (8 NeuronCores).

Problem: B=128 decode batch, HK=HV=32 heads, D=128 head dim, K=4 causal conv.
  1. per-channel causal conv1d update + silu over mixed_qkv (12288 channels)
  2. split q/k/v, l2norm(q)*D^-0.5, l2norm(k)
  3. fused KDA gate g = -exp(A_log)*softplus(forget_gate + dt_bias), b=sigmoid(beta)
  4. gated delta-rule readout:
       S' = S * exp(g);  kv = k @ S';  delta = (v - kv)*b
       o  = q @ (S' + k (x) delta) = q @ S' + (q.k) * delta
     The updated state is never materialized; with qk = qhat.khat, cc = qk*b:
       o = ((qhat - cc*khat)*eg) @ S + cc * v.

Sharding: data-parallel over batch — 16 batches per core, all 32 heads, zero
cross-core communication.

Memory-bound on the ssm_state stream; everything is organized so the DMA
engines stream the state uninterrupted end to end:
  - ssm_state ships as fp16 (2 B/elem, ~2^-11 relative quantization),
    host-pre-transposed to [k, b, h, v] so every chunk DMA reads 16 KB
    contiguous per partition (line-rate descriptors).
  - ALL 8 chunks get their own SBUF buffer (spool bufs=8, ~128 KB/partition)
    so chunk DMAs are never gated on the consumer — the prologue latency
    hides entirely under the stream.
  - conv window inputs ship fp16 in the compute layout [d, (sec, h, b)];
    conv weights / gate biases ship compact and broadcast on-chip with
    stride-0 APs.
  - the prologue uses no 1-lane row ops, no DVE reciprocal, and only 3 ACT
    table loads: partition reductions are all-ones 128x128 fp16 stationary
    matmuls that sum AND broadcast in one shot; rsqrt(x) = exp(-0.5 ln x);
    silu/sigmoid are built from the tanh entry of the exp table (scale
    factors folded into downstream constants).
  - per (b,h): ONE PE matmul — stationary S[b,h] (fp16 fast-weight-load),
    moving mg (fp16, N=1) into a per-chunk PSUM tile drained by one fused
    DVE op per chunk.
"""

import numpy as np

import concourse.bass as bass
import concourse.bacc as bacc
import concourse.mybir as mybir
from concourse.tile import TileContext
from concourse.bass_utils import run_bass_kernel_spmd

F32 = mybir.dt.float32
F16 = mybir.dt.float16
AF = mybir.ActivationFunctionType
OP = mybir.AluOpType

NCORES = 8
B, HK, HV, D, CK = 128, 32, 32, 128, 4
SEC = 3                      # q | k | v channel sections of 32 heads each
BC = B // NCORES             # batches per core = 16
NHB = HV * BC                # free columns per section = 512
G = SEC * HV                 # (sec, h) groups = 96
QKV = (2 * HK + HV) * D      # 12288
CB = 1                       # batches per ssm chunk
NCH = BC // CB               # chunks = 16

_CACHE = {}


def _build_nc():
    # Bacc (not raw Bass): its compile() splits multi-sem waits into event
    # semaphores — TRN2 instructions carry at most one wait.
    nc = bacc.Bacc("TRN2", target_bir_lowering=False, debug=False)
    cst = nc.declare_dram_parameter("cst", [CK - 1, D, G, BC], F16, isOutput=False)
    xq = nc.declare_dram_parameter("xq", [D, G, BC], F16, isOutput=False)
    wc = nc.declare_dram_parameter("wc", [D, CK, G], F16, isOutput=False)
    fg = nc.declare_dram_parameter("fg", [D, HV, BC], F16, isOutput=False)
    dtb = nc.declare_dram_parameter("dtb", [D, HV], F32, isOutput=False)
    nega = nc.declare_dram_parameter("nega", [D, HV], F32, isOutput=False)
    betar = nc.declare_dram_parameter("betar", [1, NHB], F16, isOutput=False)
    # ssm pre-transposed on host to [k, b, h, v], fp16
    ssm = nc.declare_dram_parameter("ssm", [D, BC, HV, D], F16, isOutput=False)
    o_out = nc.declare_dram_parameter("o_out", [D, BC * HV], F32, isOutput=True)

    HLN = -0.5 * float(np.log(float(D)))  # fold D**-0.5 into the q rsqrt

    with TileContext(nc) as tc:
        with (
            tc.tile_pool(name="const", bufs=1) as const,
            tc.tile_pool(name="work", bufs=1) as work,
            tc.tile_pool(name="spool", bufs=NCH) as spool,
            tc.tile_pool(name="psb", bufs=1, space="PSUM") as psb,
            tc.tile_pool(name="pso", bufs=2, space="PSUM") as pso,
        ):
            # ---- input staging ------------------------------------------
            # ALL consts on the scalar HWDGE ring so the sync ring starts
            # streaming ssm chunks immediately after the kernel barrier.
            t_w = const.tile([D, CK, G], F16)
            nc.scalar.dma_start(t_w[:], wc[:])
            t_cst = const.tile([D, CK - 1, G, BC], F16)
            for j in range(CK - 1):
                nc.scalar.dma_start(t_cst[:, j], cst[:][j])
            t_xq = const.tile([D, G, BC], F16)
            nc.scalar.dma_start(t_xq[:], xq[:])

            t_dtb = const.tile([D, HV], F32)
            nc.scalar.dma_start(t_dtb[:], dtb[:])
            t_nega = const.tile([D, HV], F32)
            nc.scalar.dma_start(t_nega[:], nega[:])
            t_beta = const.tile([1, NHB], F16)
            nc.scalar.dma_start(t_beta[:], betar[:])
            t_fg = const.tile([D, HV, BC], F16)
            nc.scalar.dma_start(t_fg[:], fg[:])

            ones_dd = const.tile([D, D], F16)
            nc.vector.memset(ones_dd[:], 1.0)
            ones_r = const.tile([1, D], F16)
            nc.vector.memset(ones_r[:], 1.0)
            hln_c = const.tile([D, 1], F32)
            nc.vector.memset(hln_c[:], HLN)

            def bc_b(ap, n=BC):
                # broadcast a [D, ...] AP along a trailing batch dim
                return ap.unsqueeze(ap.ndim).broadcast_to(tuple(ap.shape) + (n,))

            # ---- KDA gate input: ez = exp(fg + dt_bias) (exp table) -------
            g1 = work.tile([D, HV, BC], F32)
            nc.vector.tensor_tensor(g1[:], t_fg[:], bc_b(t_dtb[:]), OP.add)
            ez = work.tile([D, HV, BC], F32)
            nc.scalar.activation(ez[:], g1[:], AF.Exp)

            # ---- b = sigmoid(beta) = 0.5*tanh(beta/2)+0.5 (tanh is in the -
            # exp table; no extra load)
            bb_ps = psb.tile([D, NHB], F32)
            nc.tensor.matmul(bb_ps[:], ones_r[:], t_beta[:], start=True, stop=True)
            bsig = work.tile([D, NHB], F32)
            nc.scalar.activation(bsig[:], bb_ps[:], AF.Tanh, scale=0.5)
            nc.scalar.activation(bsig[:], bsig[:], AF.Copy, scale=0.5, bias=0.5)

            # ---- causal conv1d single-step ------------------------------
            # (gpsimd only for the two muls whose latency hides under the
            # vector muls; all adds on vector)
            acc = work.tile([D, G, BC], F16)
            t1 = work.tile([D, G, BC], F16)
            t2 = work.tile([D, G, BC], F16)
            t3 = work.tile([D, G, BC], F16)
            nc.vector.tensor_tensor(acc[:], t_cst[:, 0], bc_b(t_w[:, 0]), OP.mult)
            nc.gpsimd.tensor_tensor(t1[:], t_cst[:, 1], bc_b(t_w[:, 1]), OP.mult)
            nc.vector.tensor_tensor(t2[:], t_cst[:, 2], bc_b(t_w[:, 2]), OP.mult)
            nc.gpsimd.tensor_tensor(t3[:], t_xq[:], bc_b(t_w[:, CK - 1]), OP.mult)
            nc.vector.tensor_tensor(acc[:], acc[:], t1[:], OP.add)
            nc.vector.tensor_tensor(acc[:], acc[:], t2[:], OP.add)
            nc.vector.tensor_tensor(acc[:], acc[:], t3[:], OP.add)
            # silu via the exp-table tanh: 2*silu(a) = a*(1+tanh(a/2)).
            # x2 = 2*[q|k|v]; the factor 2 cancels in the l2 norms and is
            # folded into the epilogue's 0.5 for the v term.
            th = work.tile([D, G, BC], F32)
            nc.scalar.activation(th[:], acc[:], AF.Tanh, scale=0.5)
            x2 = work.tile([D, SEC * NHB], F16)
            nc.vector.scalar_tensor_tensor(
                x2[:], th[:].rearrange("p a b -> p (a b)"), 1.0,
                acc[:].rearrange("p a b -> p (a b)"), OP.add, OP.mult)
            q2 = x2[:, 0:NHB]
            k2 = x2[:, NHB:2 * NHB]
            v2 = x2[:, 2 * NHB:3 * NHB]

            # ---- l2 norms: fp16 all-ones matmul sums + broadcasts; -------
            # rsqrt via exp(-0.5 ln x) on the exp/ln tables
            sq = work.tile([D, 2 * NHB], F16)
            nc.vector.tensor_tensor(sq[:, 0:NHB], q2, q2, OP.mult)
            nc.vector.tensor_tensor(sq[:, NHB:2 * NHB], k2, k2, OP.mult)
            nb = psb.tile([D, 2 * NHB], F32)
            nc.tensor.matmul(nb[:, 0:NHB], ones_dd[:], sq[:, 0:NHB],
                             start=True, stop=True)
            nc.tensor.matmul(nb[:, NHB:2 * NHB], ones_dd[:], sq[:, NHB:2 * NHB],
                             start=True, stop=True)
            # ln group (one table switch for all ln uses)
            sp = work.tile([D, HV, BC], F32)
            nc.scalar.activation(sp[:], ez[:], AF.Ln, bias=1.0)  # softplus
            rb = work.tile([D, 2 * NHB], F32)
            nc.scalar.activation(rb[:], nb[:], AF.Ln)
            # back to the exp table for the rest
            nc.scalar.activation(rb[:, 0:NHB], rb[:, 0:NHB], AF.Exp,
                                 scale=-0.5, bias=hln_c[:])
            nc.scalar.activation(rb[:, NHB:2 * NHB], rb[:, NHB:2 * NHB],
                                 AF.Exp, scale=-0.5)
            g2 = work.tile([D, HV, BC], F32)
            nc.vector.tensor_tensor(g2[:], sp[:], bc_b(t_nega[:]), OP.mult)
            eg = work.tile([D, NHB], F32)
            nc.scalar.activation(eg[:], g2[:].rearrange("p a b -> p (a b)"),
                                 AF.Exp)

            qh = work.tile([D, NHB], F32)
            nc.vector.tensor_tensor(qh[:], q2, rb[:, 0:NHB], OP.mult)
            kh = work.tile([D, NHB], F32)
            nc.vector.tensor_tensor(kh[:], k2, rb[:, NHB:2 * NHB], OP.mult)

            # ---- qk = qhat.khat, broadcast via ones-matmul ---------------
            sqk = work.tile([D, NHB], F16)
            nc.vector.tensor_tensor(sqk[:], qh[:], kh[:], OP.mult)
            qkb_ps = psb.tile([D, NHB], F32)
            nc.tensor.matmul(qkb_ps[:], ones_dd[:], sqk[:], start=True, stop=True)
            cc = work.tile([D, NHB], F32)
            nc.vector.tensor_tensor(cc[:], qkb_ps[:], bsig[:], OP.mult)

            # ---- fold the delta-rule correction into one query vector ----
            # mg = (qhat - cc*khat) * eg ; cv = cc * v2 (in [d, b, h] layout)
            cv = work.tile([D, BC, HV], F32)
            nc.vector.tensor_tensor(
                cv[:], cc[:].rearrange("p (h b) -> p b h", b=BC),
                v2.rearrange("p (h b) -> p b h", b=BC), OP.mult)
            mg = work.tile([D, NHB], F32)
            nc.vector.tensor_tensor(mg[:], cc[:], kh[:], OP.mult)
            nc.vector.tensor_tensor(mg[:], qh[:], mg[:], OP.subtract)
            nc.vector.tensor_tensor(mg[:], mg[:], eg[:], OP.mult)
            mgh = work.tile([D, NHB], F16)
            nc.vector.tensor_copy(mgh[:], mg[:])

            # ---- main loop: stream S, one mat-vec per (b,h) --------------
            sr = ssm[:].rearrange("k c h v -> c k h v")
            o_v = o_out[:].rearrange("k (c h) -> c k h", h=HV)
            o_t = work.tile([D, BC, HV], F32)
            for c in range(NCH):
                Sh = spool.tile([D, HV, D], F16, name="Sh", tag="Sh")
                nc.sync.dma_start(Sh[:], sr[c])
                T = pso.tile([D, HV], F32, name="T", tag="T")
                for h in range(HV):
                    nc.tensor.matmul(
                        T[:, h:h + 1], Sh[:, h, :],
                        mgh[:, h * BC + c:h * BC + c + 1],
                        start=True, stop=True)
                # o = mg@S + 0.5*cc*v2  (0.5 undoes the doubled silu in v2)
                nc.vector.scalar_tensor_tensor(
                    o_t[:, c], cv[:, c], 0.5, T[:], OP.mult, OP.add)
                # per-batch output writeback on the gpsimd (SWDGE) ring
                nc.gpsimd.dma_start(o_v[c], o_t[:, c])

    nc.compile()
    return nc


def _prep_act(a):
    """[bc, sec*32*128] activation slice -> [128 d, sec*32, bc] fp16."""
    bcn = a.shape[0]
    return np.ascontiguousarray(
        a.reshape(bcn, G, D).transpose(2, 1, 0)).astype(np.float16)


def _prep_inputs(mixed_qkv, forget_gate, beta, conv_state, conv_weights,
                 ssm_state, A_log, dt_bias):
    mixed_qkv = np.asarray(mixed_qkv, np.float32)
    forget_gate = np.asarray(forget_gate, np.float32)
    beta = np.asarray(beta, np.float32)
    conv_state = np.asarray(conv_state, np.float32)
    conv_weights = np.asarray(conv_weights, np.float32)
    ssm_state = np.asarray(ssm_state, np.float32)
    A_log = np.asarray(A_log, np.float32)
    dt_bias = np.asarray(dt_bias, np.float32)

    # shared (weight) tensors
    wr = conv_weights.reshape(SEC, HV, D, CK).transpose(3, 2, 0, 1)  # [4,d,sec,h]
    wcp = np.ascontiguousarray(
        wr.transpose(1, 0, 2, 3).reshape(D, CK, G)).astype(np.float16)
    dtb = np.ascontiguousarray(dt_bias.reshape(HV, D).T)             # [D, HV]
    negv = np.ascontiguousarray(
        np.broadcast_to((-np.exp(A_log))[None, :], (D, HV)))

    in_maps = []
    for c in range(NCORES):
        cs = slice(c * BC, (c + 1) * BC)
        cstc = conv_state[cs]  # [BC, QKV, 3]
        cstp = np.stack([_prep_act(cstc[:, :, j]) for j in range(CK - 1)],
                        axis=0)  # [3, D, G, BC]
        fgp = np.ascontiguousarray(
            forget_gate[cs].reshape(BC, HV, D).transpose(2, 1, 0)
        ).astype(np.float16)                                         # [D,HV,BC]
        betar = np.ascontiguousarray(
            beta[cs].T.reshape(1, NHB)).astype(np.float16)           # (h,b)
        ssm_c = np.ascontiguousarray(
            ssm_state[cs].astype(np.float16).transpose(2, 0, 1, 3))  # [k,b,h,v]
        in_maps.append({
            "cst": np.ascontiguousarray(cstp),
            "xq": _prep_act(mixed_qkv[cs]),
            "wc": wcp,
            "fg": fgp,
            "dtb": dtb,
            "nega": negv,
            "betar": betar,
            "ssm": ssm_c,
        })
    return in_maps


def run(trace=False, **inputs):
    if "nc" not in _CACHE:
        _CACHE["nc"] = _build_nc()
    nc = _CACHE["nc"]
    in_maps = _prep_inputs(**inputs)
    res = run_bass_kernel_spmd(nc, in_maps, list(range(NCORES)), trace=trace)
    outs = []
    for c in range(NCORES):
        oc = np.asarray(res.results[c]["o_out"])  # [128, 512] in (d, b, h)
        outs.append(oc.reshape(D, BC, HV).transpose(1, 2, 0))  # [BC, HV, D]
    return np.concatenate(outs, axis=0), res


def kernel(**inputs) -> np.ndarray:
    out, _ = run(trace=False, **inputs)
    return out
